# revision 1
# baseline (speedup 1.0000x reference)
"""GCN block (edge-dropout GCN conv + BatchNorm + node dropout) on 8 Trainium2
NeuronCores.

Strategy (SPMD, one program on cores 0-7):
  - Nodes padded to NPAD = 8*49*128 = 50176; core c owns dst nodes
    [c*6272, (c+1)*6272) as 49 windows of 128.
  - Every core computes the full hn table: hn = (features @ W) * rsqrt(max(
    out_degree, 1)), written to a DRAM table [NPAD, 128] (out-degrees are
    computed sharded by src range and AllGather'd).
  - Edges are sharded by dst owner on the host; per (window, src-half) runs
    feed dma_gather (int16 indices, 512B bf16 rows spread over 4 SWDGE
    queues) and a one-hot bf16 matmul performs the segment-sum into PSUM.
    hn rows carry a fused ones column, so the same N=129 matmul also
    accumulates the kept in-degree (no separate degree matmuls).
  - BatchNorm stats via ones-lhsT matmuls accumulated over windows, then a
    tiny AllReduce; finale applies (agg-mean)*istd*gamma+beta and the node
    dropout mask.

Host-side work is limited to sharding/layout: sorting edges by owner/window,
index tables, padding, and transposing `features` (pure layout transforms).
"""

import sys

import numpy as np

for _p in ("/opt/trn_rl_repo", "/opt/pypackages"):
    if _p not in sys.path:
        sys.path.append(_p)

import concourse.bacc as bacc
import concourse.bass as bass
import concourse.mybir as mybir
import concourse.tile as tile
from concourse import library_config
from concourse.bass import _add_dep_helper
from concourse.bass_utils import run_bass_kernel_spmd

F32 = mybir.dt.float32
BF16 = mybir.dt.bfloat16
I16 = mybir.dt.int16
I32 = mybir.dt.int32
AF = mybir.ActivationFunctionType
OP = mybir.AluOpType

N_NODES = 50000
IN_FEAT = 256
OUT_FEAT = 128
P_EDGE = 0.2
P_NODE = 0.1
BN_EPS = 1e-5
CORES = 8
NPAD = 50176  # 8 * 49 * 128
LO_ROWS = 32768  # int16 index limit for the low gather range


def _r128(x):
    return (int(x) + 127) // 128 * 128


def _wrap16(flat, reps=8):
    """[L] -> [16*reps, L//16]: element j at row j%16 (replicated), col j//16."""
    a = flat.reshape(-1, 16).T  # [16, L//16]
    return np.tile(a, (reps, 1))


def prep_inputs(features, W, gamma, beta, src, dst, edge_rand, node_rand,
                n_nodes=N_NODES, npad=NPAD, lo_rows=LO_ROWS):
    """Host-side sharding/layout. Returns (shapes, per_core_input_maps)."""
    cores = CORES
    npc = npad // cores
    nw = npc // 128
    fin = features.shape[1]

    src = np.asarray(src).astype(np.int64)
    dst = np.asarray(dst).astype(np.int64)
    er = np.asarray(edge_rand).astype(np.float32)

    # ---------- dst shard: (owner core, window, src-half) ----------
    d_owner = dst // npc
    d_win = (dst % npc) // 128
    half = (src >= lo_rows).astype(np.int64)
    nseg = nw * 2

    per_core = []
    cnt = np.zeros((cores, nw, 2), np.int64)
    for c in range(cores):
        m = d_owner == c
        s_c, d_c, e_c = src[m], dst[m], er[m]
        key = (d_c % npc) // 128 * 2 + (s_c >= lo_rows)
        o = np.argsort(key, kind="stable")
        s_c, d_c, e_c, key = s_c[o], d_c[o], e_c[o], key[o]
        cc = np.bincount(key, minlength=nseg)
        cnt[c] = cc.reshape(nw, 2)
        per_core.append((s_c, d_c, e_c, key, cc))

    caps = np.zeros((nw, 2), np.int64)
    for w in range(nw):
        for r in range(2):
            mx = cnt[:, w, r].max()
            caps[w, r] = _r128(mx) if mx > 0 else 0
    # group-major global layout: for each group of GWIN windows, all lo
    # segments then all hi segments.  seg id = w*2 + r.
    GWIN = 2
    groups_w = [list(range(g, min(g + GWIN, nw)))
                for g in range(0, nw, GWIN)]
    seg_order = []
    for ws in groups_w:
        for r in range(2):
            for w in ws:
                seg_order.append(w * 2 + r)
    seg_off = np.zeros(nseg + 1, np.int64)
    off = 0
    seg_off_map = np.zeros(nseg, np.int64)
    for sid in seg_order:
        seg_off_map[sid] = off
        off += caps.reshape(-1)[sid]
    totcap = int(off)
    nch_d = totcap // 128

    # ---------- src shard (out-degree counting): (owner core, window) ----------
    s_owner = src // npc
    s_win = (src % npc) // 128
    scnt = np.zeros((cores, nw), np.int64)
    per_core_s = []
    for c in range(cores):
        m = s_owner == c
        s_c, e_c = src[m], er[m]
        key = (s_c % npc) // 128
        o = np.argsort(key, kind="stable")
        s_c, e_c, key = s_c[o], e_c[o], key[o]
        cc = np.bincount(key, minlength=nw)
        scnt[c] = cc
        per_core_s.append((s_c, e_c, key, cc))

    scaps = np.array([_r128(scnt[:, w].max()) if scnt[:, w].max() > 0 else 0
                      for w in range(nw)], np.int64)
    soff = np.concatenate([[0], np.cumsum(scaps)])
    stot = int(soff[-1])
    nch_s = stot // 128

    # ---------- shared constant inputs ----------
    bf16 = np.dtype("bfloat16")
    featT_full = np.zeros((fin, npad), np.float32)
    featT_full[:, :n_nodes] = np.asarray(features).astype(np.float32).T
    half = npad // 2
    featT_halves = [featT_full[:, :half].astype(bf16),
                    featT_full[:, half:].astype(bf16)]
    iota32 = np.broadcast_to(
        np.arange(128, dtype=np.float32), (128, 128)).copy()
    iota16 = iota32.astype(np.dtype("bfloat16"))
    ident = np.eye(128, dtype=np.float32)
    ones_row = np.ones((1, 128), np.float32)
    gam = np.asarray(gamma).astype(np.float32).reshape(1, OUT_FEAT)
    bet = np.asarray(beta).astype(np.float32).reshape(1, OUT_FEAT)
    nrand = np.asarray(node_rand).astype(np.float32)

    in_maps = []
    for c in range(cores):
        s_c, d_c, e_c, key, cc = per_core[c]
        # data positions within sorted arrays, capacity positions in padded
        data_off = np.concatenate([[0], np.cumsum(cc)])
        pos_in_seg = np.arange(len(s_c)) - data_off[key]
        tgt = seg_off_map[key] + pos_in_seg

        # pad slots hold a VALID index (row 0 of the range) so every gather
        # writes its full capacity; dstl=-1 kills their contribution.
        idxf = np.zeros(max(totcap, 1), np.int64)
        dstlf = np.full(max(nch_d * 128, 1), -1.0, np.float32)
        erf = np.zeros(max(nch_d * 128, 1), np.float32)
        w_of = (d_c % npc) // 128
        lidx = np.where(s_c >= lo_rows, s_c - lo_rows, s_c)
        idxf[tgt] = lidx
        dstlf[tgt] = (d_c % npc) - w_of * 128
        erf[tgt] = e_c
        if len(lidx):
            assert int(lidx.max()) < max(lo_rows, npad - lo_rows)
        idx16 = _wrap16(idxf.astype(np.int16))
        dstl_t = np.ascontiguousarray(dstlf.reshape(-1, 128).T)
        er_t = np.ascontiguousarray(erf.reshape(-1, 128).T)

        # src shard tables
        s_s, e_s, skey, scc = per_core_s[c]
        sdata_off = np.concatenate([[0], np.cumsum(scc)])
        spos = np.arange(len(s_s)) - sdata_off[skey]
        stgt = soff[skey] + spos
        srclf = np.full(max(stot, 1), -1.0, np.float32)
        ersf = np.zeros(max(stot, 1), np.float32)
        srclf[stgt] = (s_s % npc) - skey * 128
        ersf[stgt] = e_s
        srcl_t = np.ascontiguousarray(srclf.reshape(-1, 128).T)
        ers_t = np.ascontiguousarray(ersf.reshape(-1, 128).T)

        nr = np.ones((npc, OUT_FEAT), np.float32)
        lo_n = c * npc
        hi_n = min((c + 1) * npc, n_nodes)
        if hi_n > lo_n:
            nr[: hi_n - lo_n] = nrand[lo_n:hi_n]

        in_maps.append({
            "featT": featT_halves[c % 2],
            "w_mat": np.asarray(W).astype(np.float32).astype(bf16),
            "gam": gam, "bet": bet, "iota16": iota16,
            "ident": ident, "ones_row": ones_row,
            "idx16": idx16, "dstl": dstl_t, "erd": er_t,
            "srcl": srcl_t, "ers": ers_t,
            "noder": nr,
        })

    shapes = dict(npad=npad, npc=npc, nw=nw, fin=fin, lo_rows=lo_rows,
                  nch_d=max(nch_d, 1), nch_s=max(nch_s, 1),
                  totcap=max(totcap, 1), stot=max(stot, 1),
                  caps=caps, scaps=scaps, seg_off_map=seg_off_map, soff=soff,
                  groups_w=groups_w, n_nodes=n_nodes)
    return shapes, in_maps


def build_program(sh, cut=None, nocc=False, repeat=1, msk_pre=False,
                  act_off=False):
    npad, npc, nw, fin = sh["npad"], sh["npc"], sh["nw"], sh["fin"]
    lo_rows = sh["lo_rows"]
    caps, scaps = sh["caps"], sh["scaps"]
    seg_off_map, soff = sh["seg_off_map"], sh["soff"]
    groups_w = sh["groups_w"]
    n_nodes = sh["n_nodes"]
    nt = npad // 128          # node tiles
    kt = fin // 128           # contraction tiles for features @ W

    nc = bacc.Bacc("TRN2", target_bir_lowering=False, debug=False,
                   num_devices=CORES, num_swdge_queues=4)

    featT = nc.dram_tensor("featT", [fin, npad // 2], BF16,
                           kind="ExternalInput")
    w_mat = nc.dram_tensor("w_mat", [fin, OUT_FEAT], BF16, kind="ExternalInput")
    gam = nc.dram_tensor("gam", [1, OUT_FEAT], F32, kind="ExternalInput")
    bet = nc.dram_tensor("bet", [1, OUT_FEAT], F32, kind="ExternalInput")
    iota16 = nc.dram_tensor("iota16", [128, 128], BF16, kind="ExternalInput")
    ident = nc.dram_tensor("ident", [128, 128], F32, kind="ExternalInput")
    ones_row = nc.dram_tensor("ones_row", [1, 128], F32, kind="ExternalInput")
    idx16 = nc.dram_tensor("idx16", [128, sh["totcap"] // 16], I16,
                           kind="ExternalInput")
    dstl = nc.dram_tensor("dstl", [128, sh["nch_d"]], F32, kind="ExternalInput")
    erd = nc.dram_tensor("erd", [128, sh["nch_d"]], F32, kind="ExternalInput")
    srcl = nc.dram_tensor("srcl", [128, sh["nch_s"]], F32, kind="ExternalInput")
    ers = nc.dram_tensor("ers", [128, sh["nch_s"]], F32, kind="ExternalInput")
    noder = nc.dram_tensor("noder", [npc, OUT_FEAT], F32, kind="ExternalInput")
    out = nc.dram_tensor("out", [npc, OUT_FEAT], F32, kind="ExternalOutput")

    # hn rows are 256 wide (512B): features in 0:128, ones at col 128 (for
    # fused in-degree), cols 129:256 never read.
    HNW = 2 * OUT_FEAT
    hn = nc.dram_tensor("hn", [npad, HNW], BF16, addr_space="Shared")
    barr_in = nc.dram_tensor("barr_in", [1, 128], F32)
    barr_out = nc.dram_tensor("barr_out", [1, 128], F32)
    degb_in = nc.dram_tensor("degb_in", [1, npc], F32)
    degb_out = nc.dram_tensor("degb_out", [nt, 128], F32)
    statb_in = nc.dram_tensor("statb_in", [1, 2 * OUT_FEAT], F32)
    statb_out = nc.dram_tensor("statb_out", [1, 2 * OUT_FEAT], F32)

    groups = [list(range(CORES))]
    pair_groups = [[2 * i, 2 * i + 1] for i in range(CORES // 2)]

    with tile.TileContext(nc) as tc:
        nc.gpsimd.load_library(library_config.mlp)
        with (
            tc.tile_pool(name="const", bufs=1) as cpool,
            tc.tile_pool(name="aux", bufs=1) as apool,
            tc.tile_pool(name="work", bufs=1) as wpool,
            tc.tile_pool(name="psum", bufs=1, space="PSUM") as pps,
        ):
            # ---------- constants ----------
            # load W as kt tiles of [128, OUT]
            w_tiles = []
            for k in range(kt):
                wt = cpool.tile([128, OUT_FEAT], BF16, tag=f"wk{k}", name=f"wk{k}")
                nc.sync.dma_start(out=wt[:, :], in_=w_mat[k * 128:(k + 1) * 128, :])
                w_tiles.append(wt)
            io16 = cpool.tile([128, 128], BF16, tag="io16", name="io16")
            nc.sync.dma_start(out=io16[:, :], in_=iota16[:, :])
            idn = cpool.tile([128, 128], F32, tag="idn", name="idn")
            nc.sync.dma_start(out=idn[:, :], in_=ident[:, :])
            onesr = cpool.tile([1, 128], F32, tag="onesr", name="onesr")
            nc.sync.dma_start(out=onesr[:, :], in_=ones_row[:, :])
            ones16 = cpool.tile([128, 1], BF16, tag="ones16", name="ones16")
            nc.vector.memset(ones16[:, :], 1.0)
            gam_sb = cpool.tile([1, OUT_FEAT], F32, tag="gam_sb", name="gam_sb")
            nc.sync.dma_start(out=gam_sb[:, :], in_=gam[:, :])
            bet_sb = cpool.tile([1, OUT_FEAT], F32, tag="bet_sb", name="bet_sb")
            nc.sync.dma_start(out=bet_sb[:, :], in_=bet[:, :])

            for _rep in range(repeat):
                do_b = cut in (None, "B", "C", "D")
                do_c = cut in (None, "C", "D")
                do_d = cut in (None, "D")
                do_e = cut is None
                if do_b:
                    # ---------- phase B: out-degree (deg_src) ----------
                    srcl_sb = apool.tile([128, sh["nch_s"]], F32, tag="srcl_sb",
                                         name="srcl_sb")
                    nc.sync.dma_start(out=srcl_sb[:, :], in_=srcl[:, :])
                    ers_sb = apool.tile([128, sh["nch_s"]], F32, tag="ers_sb",
                                        name="ers_sb")
                    nc.sync.dma_start(out=ers_sb[:, :], in_=ers[:, :])
                    keep_s = apool.tile([128, sh["nch_s"]], F32, tag="keep_s",
                                        name="keep_s")
                    nc.vector.tensor_scalar(keep_s[:, :], ers_sb[:, :], P_EDGE, None,
                                            op0=OP.is_ge)
                    if act_off:
                        # negated copies for ACT-engine one-hot builds
                        nsrcl = apool.tile([128, sh["nch_s"]], F32, tag="nsrcl",
                                           name="nsrcl")
                        nc.vector.tensor_scalar(nsrcl[:, :], srcl_sb[:, :],
                                                -1.0, None, op0=OP.mult)
                        nkeep = apool.tile([128, sh["nch_s"]], F32, tag="nkeep",
                                           name="nkeep")
                        nc.vector.tensor_scalar(nkeep[:, :], keep_s[:, :],
                                                -1.0, None, op0=OP.mult)

                    # staging row for this core's deg slice
                    degrow = wpool.tile([1, npc], F32, tag="degrow", name="degrow")

                    # quad-batched: 4 chunks' one-hots side by side in one
                    # [128, 512] tile, one matmul per quad (PE cost is per
                    # instruction, nearly independent of N).
                    with tc.tile_pool(name="degb_ps", bufs=4, space="PSUM") as dps_pool, \
                            tc.tile_pool(name="msrc", bufs=6) as mpool_s, \
                            tc.tile_pool(name="dfold", bufs=4) as df_pool:
                        for w in range(nw):
                            ncap = int(scaps[w]) // 128
                            if ncap == 0:
                                nc.vector.memset(
                                    degrow[0:1, w * 128:(w + 1) * 128], 0.0)
                                continue
                            base = int(soff[w]) // 128
                            nquad = (ncap + 3) // 4
                            dps = dps_pool.tile([1, 512], F32, tag="dps",
                                                name=f"dps{w}")
                            for qi in range(nquad):
                                wq = min(4, ncap - qi * 4)
                                m4 = mpool_s.tile([128, 512], BF16, tag="m4",
                                                  name=f"m4_{w}_{qi}")
                                for j in range(wq):
                                    col = base + qi * 4 + j
                                    m_sl = m4[:, j * 128:(j + 1) * 128]
                                    if act_off and j == 3:
                                        # exact one-hot on ACT: t=|io-srcl|,
                                        # relu(keep - keep*t)
                                        tt = mpool_s.tile(
                                            [128, 128], BF16, tag="tt",
                                            name=f"tt_{w}_{qi}")
                                        nc.scalar.activation(
                                            tt[:, :], io16[:, :], AF.Abs,
                                            bias=nsrcl[:, col:col + 1])
                                        nc.scalar.activation(
                                            m_sl, tt[:, :], AF.Relu,
                                            bias=keep_s[:, col:col + 1],
                                            scale=nkeep[:, col:col + 1])
                                    else:
                                        nc.vector.tensor_scalar(
                                            m_sl, io16[:, :],
                                            srcl_sb[:, col:col + 1],
                                            keep_s[:, col:col + 1],
                                            op0=OP.is_equal, op1=OP.mult)
                                if wq < 4:
                                    nc.vector.memset(m4[:, wq * 128:512], 0.0)
                                nc.tensor.matmul(
                                    dps[0:1, :],
                                    lhsT=ones16[:, :], rhs=m4[:, :],
                                    start=(qi == 0), stop=(qi == nquad - 1))
                            # fold <=4 sub-blocks into the window's deg row
                            # (DVE may read at most one PSUM input: stage via
                            # an SBUF copy first)
                            nblk = min(ncap, 4)
                            dslice = degrow[0:1, w * 128:(w + 1) * 128]
                            if nblk == 1:
                                nc.scalar.copy(dslice, dps[0:1, 0:128])
                            else:
                                fs = df_pool.tile([1, 512], F32, tag="fs",
                                                  name=f"fs{w}")
                                nc.scalar.copy(fs[0:1, 0:nblk * 128],
                                               dps[0:1, 0:nblk * 128])
                                if nblk == 2:
                                    nc.vector.tensor_add(
                                        dslice, fs[0:1, 0:128], fs[0:1, 128:256])
                                elif nblk == 3:
                                    nc.vector.tensor_add(
                                        fs[0:1, 0:128], fs[0:1, 0:128],
                                        fs[0:1, 128:256])
                                    nc.vector.tensor_add(
                                        dslice, fs[0:1, 0:128], fs[0:1, 256:384])
                                else:
                                    nc.vector.tensor_add(
                                        fs[0:1, 0:256], fs[0:1, 0:256],
                                        fs[0:1, 256:512])
                                    nc.vector.tensor_add(
                                        dslice, fs[0:1, 0:128], fs[0:1, 128:256])
                    nc.sync.dma_start(out=degb_in[:, :], in_=degrow[:, :])
                    if nocc:
                        nc.sync.dma_start(
                            out=degb_out[0:npc // 128, :],
                            in_=degb_in.ap().rearrange("o (r c) -> (o r) c", c=128))
                    else:
                        nc.gpsimd.collective_compute(
                            "AllGather", OP.bypass, replica_groups=groups,
                            ins=[degb_in.ap().opt()], outs=[degb_out.ap().opt()])

                    # reload MY half's degrees -> [128, nt//2] rsqrt(max(deg,1))
                    nt_half = nt // 2
                    par = nc.sync.partition_id() % 2
                    rbase = par * nt_half
                    rdegs = wpool.tile([128, nt_half], F32, tag="rdegs",
                                       name="rdegs")
                    with tc.tile_pool(name="degld", bufs=2) as dl_pool, \
                            tc.tile_pool(name="degt_ps", bufs=2, space="PSUM") as tps_pool:
                        for blk in range((nt_half + 127) // 128):
                            r0 = blk * 128
                            r1 = min(r0 + 128, nt_half)
                            nrow = r1 - r0
                            dl = dl_pool.tile([128, 128], F32, tag="dl", name=f"dl{blk}")
                            nc.sync.dma_start(
                                out=dl[0:nrow, :],
                                in_=degb_out[bass.ds(rbase + r0, nrow), :])
                            tp = tps_pool.tile([128, 128], F32, tag="tp", name=f"tp{blk}")
                            nc.tensor.transpose(tp[:, 0:nrow], dl[0:nrow, :],
                                                idn[0:nrow, 0:nrow])
                            t1 = dl_pool.tile([128, 128], F32, tag="t1", name=f"t1{blk}")
                            nc.vector.tensor_scalar_max(t1[:, 0:nrow],
                                                        tp[:, 0:nrow], 1.0)
                            t2 = dl_pool.tile([128, 128], F32, tag="t2", name=f"t2{blk}")
                            nc.scalar.sqrt(t2[:, 0:nrow], t1[:, 0:nrow])
                            nc.vector.reciprocal(rdegs[:, r0:r1], t2[:, 0:nrow])

                if do_c:
                    # ---------- phase C: hn table (my half, batched DMA) ----------
                    NB = 4
                    assert nt_half % NB == 0
                    row_base = par * (nt_half * 128)
                    hn_writes = []
                    with tc.tile_pool(name="hload", bufs=6) as hl_pool, \
                            tc.tile_pool(name="hps", bufs=4, space="PSUM") as hps_pool, \
                            tc.tile_pool(name="hout", bufs=3) as ho_pool:
                        for b in range(nt_half // NB):
                            n0 = b * NB * 128
                            ft = hl_pool.tile([128, kt * NB * 128], BF16, tag="ft",
                                              name=f"ft{b}")
                            nc.sync.dma_start(
                                out=ft[:, :].rearrange("p (k n) -> p k n", k=kt),
                                in_=featT[:, n0:n0 + NB * 128].rearrange(
                                    "(k p) n -> p k n", k=kt))
                            # 129-wide rows: 128 scaled features + a ones col
                            hnt = ho_pool.tile([128, NB * 129], BF16, tag="hnt",
                                               name=f"hnt{b}")
                            nc.vector.memset(hnt[:, :], 1.0)
                            for j in range(NB):
                                hps = hps_pool.tile([128, OUT_FEAT], F32, tag="hps",
                                                    name=f"hps{b}_{j}")
                                for k in range(kt):
                                    nc.tensor.matmul(
                                        hps[:, :],
                                        lhsT=ft[:, (k * NB + j) * 128:
                                                (k * NB + j + 1) * 128],
                                        rhs=w_tiles[k][:, :],
                                        start=(k == 0), stop=(k == kt - 1))
                                t = b * NB + j
                                nc.scalar.activation(
                                    hnt[:, j * 129:j * 129 + 128], hps[:, :],
                                    AF.Copy, scale=rdegs[:, t:t + 1])
                            wr = nc.sync.dma_start(
                                out=hn[bass.ds(row_base + n0, NB * 128),
                                       0:129].rearrange(
                                    "(t p) f -> p t f", p=128),
                                in_=hnt[:, :].rearrange("p (t f) -> p t f", t=NB))
                            hn_writes.append(wr)
                    # pair barrier: both halves of the shared table written
                    bw = nc.sync.dma_start(out=barr_in[:, :], in_=onesr[:, :])
                    for wr in hn_writes:
                        _add_dep_helper(bw.ins, wr.ins, sync=True,
                                        reason="hn writes before pair barrier")
                    if nocc:
                        barrier_cc = None
                    else:
                        barrier_cc = nc.gpsimd.collective_compute(
                            "AllReduce", OP.add, replica_groups=pair_groups,
                            ins=[barr_in.ap().opt()], outs=[barr_out.ap().opt()])

                if do_d:
                    # ---------- phase D: gather + segment-sum ----------
                    idx_sb = apool.tile([128, sh["totcap"] // 16], I16, tag="idx_sb",
                                        name="idx_sb")
                    nc.sync.dma_start(out=idx_sb[:, :], in_=idx16[:, :])
                    dstl_sb = apool.tile([128, sh["nch_d"]], F32, tag="dstl_sb",
                                         name="dstl_sb")
                    nc.sync.dma_start(out=dstl_sb[:, :], in_=dstl[:, :])
                    erd_sb = apool.tile([128, sh["nch_d"]], F32, tag="erd_sb",
                                        name="erd_sb")
                    nc.sync.dma_start(out=erd_sb[:, :], in_=erd[:, :])
                    keep_d = apool.tile([128, sh["nch_d"]], F32, tag="keep_d",
                                        name="keep_d")
                    nc.vector.tensor_scalar(keep_d[:, :], erd_sb[:, :], P_EDGE, None,
                                            op0=OP.is_ge)

                    stat_ps = pps.tile([1, 2 * OUT_FEAT], F32, tag="stat_ps",
                                       name="stat_ps")

                    # optional: build node-dropout masks here (inputs-only
                    # dependency) so they come off the post-AllReduce tail;
                    # phase D's DVE has slack under the gathers.
                    inv_keep = 1.0 / (1.0 - P_NODE)
                    msk_tiles = {}
                    if msk_pre:
                        NBF = 7
                        with tc.tile_pool(name="nrl", bufs=3) as nr_pool:
                            for b in range((nw + NBF - 1) // NBF):
                                ws_m = list(range(b * NBF,
                                                  min((b + 1) * NBF, nw)))
                                nb = len(ws_m)
                                n0 = ws_m[0] * 128
                                nrt = nr_pool.tile([128, NBF * OUT_FEAT], F32,
                                                   tag="nrt", name=f"nrt{b}")
                                nc.sync.dma_start(
                                    out=nrt[:, 0:nb * OUT_FEAT].rearrange(
                                        "p (t f) -> p t f", t=nb),
                                    in_=noder[n0:n0 + nb * 128, :].rearrange(
                                        "(t p) f -> p t f", p=128))
                                for i, w in enumerate(ws_m):
                                    msk = wpool.tile([128, OUT_FEAT], BF16,
                                                     tag=f"msk{w}",
                                                     name=f"msk{w}")
                                    nc.vector.tensor_scalar(
                                        msk[:, :],
                                        nrt[:, i * OUT_FEAT:(i + 1) * OUT_FEAT],
                                        P_NODE, inv_keep,
                                        op0=OP.is_ge, op1=OP.mult)
                                    msk_tiles[w] = msk
                    active = [w for w in range(nw) if caps[w, 0] + caps[w, 1] > 0]
                    agg_tiles = {}
                    max_gcap = max(
                        (sum(int(caps[w, 0] + caps[w, 1]) for w in ws)
                         for ws in groups_w), default=128)
                    hn_lo = hn[0:lo_rows, :]
                    hn_hi = hn[lo_rows:npad, :]

                    ngath = 0
                    with tc.tile_pool(name="gath", bufs=4) as gpool, \
                            tc.tile_pool(name="mdst", bufs=8) as mpool, \
                            tc.tile_pool(name="aggps", bufs=3, space="PSUM") as aps_pool, \
                            tc.tile_pool(name="wtmp", bufs=3) as tpool:
                        for gidx, ws in enumerate(groups_w):
                            gcap = sum(int(caps[w, 0] + caps[w, 1]) for w in ws)
                            if gcap == 0:
                                continue
                            g0 = int(min(seg_off_map[w * 2 + r]
                                         for w in ws for r in range(2)
                                         if caps[w, r] > 0))
                            gt = gpool.tile([128, max_gcap * 2], BF16, tag="gt",
                                            name=f"gt{gidx}")
                            gt3 = gt[:, 0:gcap * 2].rearrange(
                                "p (c e) -> p c e", e=2 * OUT_FEAT)
                            # one gather per (group, range), round-robin queues
                            for r, src_view in ((0, hn_lo), (1, hn_hi)):
                                rcap = sum(int(caps[w, r]) for w in ws)
                                if rcap == 0:
                                    continue
                                roff = int(min(seg_off_map[w * 2 + r] for w in ws
                                               if caps[w, r] > 0)) - g0
                                gth = nc.gpsimd.dma_gather(
                                    gt3[:, roff // 128:(roff + rcap) // 128, :],
                                    src_view,
                                    idx_sb[:, (g0 + roff) // 16:
                                           (g0 + roff + rcap) // 16],
                                    rcap, rcap, 2 * OUT_FEAT,
                                    single_packet=False,
                                    queue_num=ngath % 4)
                                ngath += 1
                                if barrier_cc is not None:
                                    _add_dep_helper(gth.ins, barrier_cc.ins,
                                                    sync=True,
                                                    reason="gather after barrier")
                            # consume chunks window-by-window; the agg matmul
                            # is 129 wide - col 128 hits the ones plane and
                            # accumulates the kept in-degree.
                            for w in ws:
                                nch_w = int(caps[w, 0] + caps[w, 1]) // 128
                                if nch_w == 0:
                                    continue
                                chunk_cols = []
                                for r in range(2):
                                    cap = int(caps[w, r])
                                    for k in range(cap // 128):
                                        chunk_cols.append(
                                            int(seg_off_map[w * 2 + r]) // 128 + k)
                                aps = aps_pool.tile([128, OUT_FEAT + 1], F32,
                                                    tag="aps", name=f"aps{w}")
                                for ki, col in enumerate(chunk_cols):
                                    mt = mpool.tile([128, 128], BF16, tag="mt",
                                                    name=f"mt{w}_{ki}")
                                    nc.vector.tensor_scalar(
                                        mt[:, :], io16[:, :],
                                        dstl_sb[:, col:col + 1],
                                        keep_d[:, col:col + 1],
                                        op0=OP.is_equal, op1=OP.mult)
                                    lo_off = (col * 128 - g0) * 2
                                    nc.tensor.matmul(
                                        aps[:, :], lhsT=mt[:, :],
                                        rhs=gt[:, lo_off:lo_off + OUT_FEAT + 1],
                                        start=(ki == 0),
                                        stop=(ki == len(chunk_cols) - 1))
                                d1 = tpool.tile([128, 1], F32, tag="d1", name=f"d1_{w}")
                                nc.vector.tensor_scalar_max(
                                    d1[:, :], aps[:, OUT_FEAT:OUT_FEAT + 1], 1.0)
                                d2 = tpool.tile([128, 1], F32, tag="d2", name=f"d2_{w}")
                                nc.scalar.sqrt(d2[:, :], d1[:, :])
                                d3 = tpool.tile([128, 1], F32, tag="d3", name=f"d3_{w}")
                                nc.vector.reciprocal(d3[:, :], d2[:, :])
                                # agg|sq packed in one tile for a single
                                # stats matmul per window
                                asq = wpool.tile([128, 2 * OUT_FEAT], BF16,
                                                 tag=f"asq{w}", name=f"asq{w}")
                                nc.scalar.activation(
                                    asq[:, 0:OUT_FEAT], aps[:, 0:OUT_FEAT],
                                    AF.Copy, scale=d3[:, :])
                                nc.scalar.square(asq[:, OUT_FEAT:2 * OUT_FEAT],
                                                 asq[:, 0:OUT_FEAT])
                                nc.tensor.matmul(stat_ps[0:1, :],
                                                 lhsT=ones16[:, :], rhs=asq[:, :],
                                                 start=(w == active[0]),
                                                 stop=(w == active[-1]))
                                agg_tiles[w] = asq

                if do_e:
                    # ---------- phase E: BN stats + finale ----------
                    stat_sb = wpool.tile([1, 2 * OUT_FEAT], F32, tag="stat_sb",
                                         name="stat_sb")
                    nc.vector.tensor_copy(stat_sb[:, :], stat_ps[:, :])
                    nc.sync.dma_start(out=statb_in[:, :], in_=stat_sb[:, :])
                    if nocc:
                        nc.sync.dma_start(out=statb_out[:, :], in_=statb_in[:, :])
                    else:
                        nc.gpsimd.collective_compute(
                            "AllReduce", OP.add, replica_groups=groups,
                            ins=[statb_in.ap().opt()], outs=[statb_out.ap().opt()])
                    stat2 = wpool.tile([1, 2 * OUT_FEAT], F32, tag="stat2",
                                       name="stat2")
                    nc.sync.dma_start(out=stat2[:, :], in_=statb_out[:, :])

                    inv_n = 1.0 / float(n_nodes)
                    mn = wpool.tile([1, OUT_FEAT], F32, tag="mn", name="mn")
                    nc.vector.tensor_scalar(mn[:, :], stat2[0:1, 0:OUT_FEAT], inv_n,
                                            None, op0=OP.mult)
                    ex2 = wpool.tile([1, OUT_FEAT], F32, tag="ex2", name="ex2")
                    nc.vector.tensor_scalar(ex2[:, :], stat2[0:1, OUT_FEAT:2 * OUT_FEAT],
                                            inv_n, None, op0=OP.mult)
                    var = wpool.tile([1, OUT_FEAT], F32, tag="var", name="var")
                    nc.vector.tensor_mul(var[:, :], mn[:, :], mn[:, :])
                    nc.vector.tensor_sub(var[:, :], ex2[:, :], var[:, :])
                    nc.vector.tensor_scalar_add(var[:, :], var[:, :], BN_EPS)
                    sd = wpool.tile([1, OUT_FEAT], F32, tag="sd", name="sd")
                    nc.scalar.sqrt(sd[:, :], var[:, :])
                    istd = wpool.tile([1, OUT_FEAT], F32, tag="istd", name="istd")
                    nc.vector.reciprocal(istd[:, :], sd[:, :])
                    st_row = wpool.tile([1, 2 * OUT_FEAT], F32, tag="st_row",
                                        name="st_row")
                    # s = gamma * istd ; t = beta - mean * s
                    nc.vector.tensor_mul(st_row[0:1, 0:OUT_FEAT], gam_sb[:, :],
                                         istd[:, :])
                    tmp_t = wpool.tile([1, OUT_FEAT], F32, tag="tmp_t", name="tmp_t")
                    nc.vector.tensor_mul(tmp_t[:, :], mn[:, :],
                                         st_row[0:1, 0:OUT_FEAT])
                    nc.vector.tensor_sub(st_row[0:1, OUT_FEAT:2 * OUT_FEAT],
                                         bet_sb[:, :], tmp_t[:, :])
                    with tc.tile_pool(name="bps", bufs=1, space="PSUM") as bps_pool:
                        bps = bps_pool.tile([128, 2 * OUT_FEAT], F32, tag="bps",
                                            name="bps")
                        nc.tensor.matmul(bps[:, :], lhsT=onesr[:, :], rhs=st_row[:, :],
                                         start=True, stop=True)
                        st_bc = wpool.tile([128, 2 * OUT_FEAT], F32, tag="st_bc",
                                           name="st_bc")
                        nc.vector.tensor_copy(st_bc[:, :], bps[:, :])

                    NBF = 7
                    with tc.tile_pool(name="fin", bufs=3) as fpool:
                        for b in range((nw + NBF - 1) // NBF):
                            ws = list(range(b * NBF, min((b + 1) * NBF, nw)))
                            nb = len(ws)
                            n0 = ws[0] * 128
                            if not msk_pre:
                                nrt = fpool.tile([128, NBF * OUT_FEAT], F32,
                                                 tag="nrt", name=f"nrt{b}")
                                nc.sync.dma_start(
                                    out=nrt[:, 0:nb * OUT_FEAT].rearrange(
                                        "p (t f) -> p t f", t=nb),
                                    in_=noder[n0:n0 + nb * 128, :].rearrange(
                                        "(t p) f -> p t f", p=128))
                            ot = fpool.tile([128, NBF * OUT_FEAT], F32, tag="ot",
                                            name=f"ot{b}")
                            for i, w in enumerate(ws):
                                agg = agg_tiles.get(w)
                                o_sl = ot[:, i * OUT_FEAT:(i + 1) * OUT_FEAT]
                                y = fpool.tile([128, OUT_FEAT], F32, tag="y",
                                               name=f"y{w}")
                                if agg is None:
                                    # no edges anywhere: agg == 0, y = t row
                                    nc.vector.tensor_copy(
                                        y[:, :], st_bc[:, OUT_FEAT:2 * OUT_FEAT])
                                else:
                                    nc.vector.tensor_mul(y[:, :],
                                                         agg[:, 0:OUT_FEAT],
                                                         st_bc[:, 0:OUT_FEAT])
                                    nc.vector.tensor_add(
                                        y[:, :], y[:, :],
                                        st_bc[:, OUT_FEAT:2 * OUT_FEAT])
                                if msk_pre:
                                    msk = msk_tiles[w]
                                else:
                                    n_sl = nrt[:, i * OUT_FEAT:(i + 1) * OUT_FEAT]
                                    msk = fpool.tile([128, OUT_FEAT], F32,
                                                     tag="msk", name=f"msk{w}")
                                    nc.vector.tensor_scalar(
                                        msk[:, :], n_sl, P_NODE, inv_keep,
                                        op0=OP.is_ge, op1=OP.mult)
                                nc.vector.tensor_mul(o_sl, y[:, :], msk[:, :])
                            nc.sync.dma_start(
                                out=out[n0:n0 + nb * 128, :].rearrange(
                                    "(t p) f -> p t f", p=128),
                                in_=ot[:, 0:nb * OUT_FEAT].rearrange(
                                    "p (t f) -> p t f", t=nb))

    nc.compile()
    return nc


_CACHE = {}


def _get_program(inputs):
    key = tuple(np.asarray(inputs["src"])[:8].tolist()) + (
        len(np.asarray(inputs["src"])),)
    if key not in _CACHE:
        sh, in_maps = prep_inputs(
            inputs["features"], inputs["W"], inputs["gamma"], inputs["beta"],
            inputs["src"], inputs["dst"], inputs["edge_rand"],
            inputs["node_rand"])
        nc = build_program(sh, msk_pre=True, act_off=True)
        _CACHE[key] = (sh, nc)
    else:
        sh, nc = _CACHE[key]
        _, in_maps = prep_inputs(
            inputs["features"], inputs["W"], inputs["gamma"], inputs["beta"],
            inputs["src"], inputs["dst"], inputs["edge_rand"],
            inputs["node_rand"])
    return sh, _CACHE[key][1], in_maps


def kernel(**inputs):
    sh, nc, in_maps = _get_program(inputs)
    res = run_bass_kernel_spmd(nc, in_maps, core_ids=list(range(CORES)))
    npc = sh["npc"]
    full = np.concatenate([res.results[c]["out"] for c in range(CORES)],
                          axis=0)
    return np.ascontiguousarray(full[:sh["n_nodes"]]).astype(np.float32)



# revision 2
# speedup vs baseline: 2.0954x; 2.0954x over previous
"""GCN block (edge-dropout GCN conv + BatchNorm + node dropout) on 8 Trainium2
NeuronCores — v2.

Differences from the v1 baseline:
  - Both degree computations (deg_src for the rsqrt-normalized hn table,
    deg_dst for the per-window scale) are fixed-slot prefix scans: the host
    lays each node's edges out in K-sized slots of a [nw*2, 64*K+1] table;
    the device does is_ge -> tensor_tensor_scan -> strided subtract.  This
    replaces ~1100 one-hot builds + ~280 matmuls with ~6 DVE instructions.
  - hn rows are 128 bf16 cols (256B) — no fused ones column.  The gather
    moves half the bytes; hnt tiles need no memset.
  - One-hot matrices for the segment-sum matmul are built 8 chunks at a time
    with a single tensor_tensor is_equal against a stride-0 broadcast of the
    keep-masked dst-local index (dm = keep*(dstl+1)-1, bf16, exact ints).
  - DMAs are split across the SP and ACT HWDGE queues; phase C scale-copies
    alternate ACT/DVE; the finale's mask multiply runs on Pool.
"""

import sys

import numpy as np

for _p in ("/opt/trn_rl_repo", "/opt/pypackages"):
    if _p not in sys.path:
        sys.path.append(_p)

import concourse.bacc as bacc
import concourse.bass as bass
import concourse.mybir as mybir
import concourse.tile as tile
from concourse import library_config
from concourse.bass import _add_dep_helper
from concourse.bass_utils import run_bass_kernel_spmd

F32 = mybir.dt.float32
BF16 = mybir.dt.bfloat16
I16 = mybir.dt.int16
AF = mybir.ActivationFunctionType
OP = mybir.AluOpType

N_NODES = 50000
IN_FEAT = 256
OUT_FEAT = 128
P_EDGE = 0.2
P_NODE = 0.1
BN_EPS = 1e-5
CORES = 8
NPAD = 50176  # 8 * 49 * 128
LO_ROWS = 32768  # int16 index limit for the low gather range
OHB = 8  # one-hot chunks built per DVE instruction


def _r128(x):
    return (int(x) + 127) // 128 * 128


def _r16(x):
    return (int(x) + 15) // 16 * 16


def _wrap16(flat, reps=8):
    """[L] -> [16*reps, L//16]: element j at row j%16 (replicated), col j//16."""
    a = flat.reshape(-1, 16).T  # [16, L//16]
    return np.tile(a, (reps, 1))


def _slot_table(local_idx, er_vals, npc, K):
    """Fixed-slot scan table: row r=local//64 holds nodes r*64..r*64+63, node
    n's edges at cols [1 + (n%64)*K, ...). Returns [npc//64, 64*K+1] f32."""
    nrows = npc // 64
    W = 64 * K + 1
    tab = np.zeros((nrows, W), np.float32)
    if len(local_idx) == 0:
        return tab
    order = np.argsort(local_idx, kind="stable")
    li = local_idx[order]
    ev = er_vals[order]
    cnt = np.bincount(li, minlength=npc)
    assert cnt.max() <= K, (cnt.max(), K)
    start = np.concatenate([[0], np.cumsum(cnt)])
    pos = np.arange(len(li)) - start[li]
    row = li // 64
    col = 1 + (li % 64) * K + pos
    tab[row, col] = ev
    return tab


def prep_inputs(features, W, gamma, beta, src, dst, edge_rand, node_rand,
                n_nodes=N_NODES, npad=NPAD, lo_rows=LO_ROWS):
    """Host-side sharding/layout. Returns (shapes, per_core_input_maps)."""
    cores = CORES
    npc = npad // cores
    nw = npc // 128
    nw2 = nw * 2
    fin = features.shape[1]

    src = np.asarray(src).astype(np.int64)
    dst = np.asarray(dst).astype(np.int64)
    er = np.asarray(edge_rand).astype(np.float32)

    # global slot sizes (uniform across cores so SPMD shapes match)
    Kd = _r16(max(1, int(np.bincount(dst, minlength=n_nodes).max())))
    Ks = _r16(max(1, int(np.bincount(src, minlength=n_nodes).max())))
    Wd = 64 * Kd + 1
    Ws = 64 * Ks + 1

    # ---------- dst shard: (owner core, window, src-half) ----------
    d_owner = dst // npc
    nseg = nw * 2

    per_core = []
    cnt = np.zeros((cores, nw, 2), np.int64)
    for c in range(cores):
        m = d_owner == c
        s_c, d_c, e_c = src[m], dst[m], er[m]
        key = (d_c % npc) // 128 * 2 + (s_c >= lo_rows)
        o = np.argsort(key, kind="stable")
        s_c, d_c, e_c, key = s_c[o], d_c[o], e_c[o], key[o]
        cc = np.bincount(key, minlength=nseg)
        cnt[c] = cc.reshape(nw, 2)
        per_core.append((s_c, d_c, e_c, key, cc))

    caps = np.zeros((nw, 2), np.int64)
    for w in range(nw):
        for r in range(2):
            mx = cnt[:, w, r].max()
            caps[w, r] = _r128(mx) if mx > 0 else 0
    # group-major global layout: for each group of GWIN windows, all lo
    # segments then all hi segments.  seg id = w*2 + r.
    GWIN = 2
    groups_w = [list(range(g, min(g + GWIN, nw)))
                for g in range(0, nw, GWIN)]
    seg_order = []
    for ws in groups_w:
        for r in range(2):
            for w in ws:
                seg_order.append(w * 2 + r)
    seg_off = np.zeros(nseg + 1, np.int64)
    off = 0
    seg_off_map = np.zeros(nseg, np.int64)
    for sid in seg_order:
        seg_off_map[sid] = off
        off += caps.reshape(-1)[sid]
    totcap = int(off)
    nch_d = totcap // 128

    # ---------- shared constant inputs ----------
    bf16 = np.dtype("bfloat16")
    featT_full = np.zeros((fin, npad), np.float32)
    featT_full[:, :n_nodes] = np.asarray(features).astype(np.float32).T
    half = npad // 2
    featT_halves = [featT_full[:, :half].astype(bf16),
                    featT_full[:, half:].astype(bf16)]
    io8 = np.tile(np.arange(128, dtype=np.float32), OHB)[None, :].repeat(
        128, axis=0).astype(bf16)
    ident = np.eye(128, dtype=np.float32)
    ones_row = np.ones((1, 128), np.float32)
    gam = np.asarray(gamma).astype(np.float32).reshape(1, OUT_FEAT)
    bet = np.asarray(beta).astype(np.float32).reshape(1, OUT_FEAT)
    nrand = np.asarray(node_rand).astype(np.float32)
    w_bf = np.asarray(W).astype(np.float32).astype(bf16)

    s_owner = src // npc

    in_maps = []
    for c in range(cores):
        s_c, d_c, e_c, key, cc = per_core[c]
        # data positions within sorted arrays, capacity positions in padded
        data_off = np.concatenate([[0], np.cumsum(cc)])
        pos_in_seg = np.arange(len(s_c)) - data_off[key]
        tgt = seg_off_map[key] + pos_in_seg

        # pad slots hold a VALID index (row 0 of the range) so every gather
        # writes its full capacity; dstl=-1 kills their contribution.
        idxf = np.zeros(max(totcap, 1), np.int64)
        dstlf = np.full(max(nch_d * 128, 1), -1.0, np.float32)
        erf = np.zeros(max(nch_d * 128, 1), np.float32)
        w_of = (d_c % npc) // 128
        lidx = np.where(s_c >= lo_rows, s_c - lo_rows, s_c)
        idxf[tgt] = lidx
        dstlf[tgt] = (d_c % npc) - w_of * 128
        erf[tgt] = e_c
        if len(lidx):
            assert int(lidx.max()) < max(lo_rows, npad - lo_rows)
        idx16 = _wrap16(idxf.astype(np.int16))
        dstl_t = np.ascontiguousarray(dstlf.reshape(-1, 128).T)
        er_t = np.ascontiguousarray(erf.reshape(-1, 128).T)

        # deg_dst scan table: this core's dst edges, slot-placed by dst-local
        dtab = _slot_table((d_c % npc).astype(np.int64), e_c, npc, Kd)

        # deg_src scan table: edges whose src is owned by this core
        ms = s_owner == c
        stab = _slot_table((src[ms] % npc).astype(np.int64), er[ms], npc, Ks)

        nr = np.ones((npc, OUT_FEAT), np.float32)
        lo_n = c * npc
        hi_n = min((c + 1) * npc, n_nodes)
        if hi_n > lo_n:
            nr[: hi_n - lo_n] = nrand[lo_n:hi_n]

        in_maps.append({
            "featT": featT_halves[c % 2],
            "w_mat": w_bf,
            "gam": gam, "bet": bet, "io8": io8,
            "ident": ident, "ones_row": ones_row,
            "idx16": idx16, "dstl": dstl_t, "erd": er_t,
            "dtab": dtab, "stab": stab,
            "noder": nr,
        })

    shapes = dict(npad=npad, npc=npc, nw=nw, nw2=nw2, fin=fin,
                  lo_rows=lo_rows, nch_d=max(nch_d, 1),
                  totcap=max(totcap, 1), Kd=Kd, Ks=Ks, Wd=Wd, Ws=Ws,
                  caps=caps, seg_off_map=seg_off_map,
                  groups_w=groups_w, n_nodes=n_nodes)
    return shapes, in_maps


def build_program(sh, nocc=False, **_ignored):
    npad, npc, nw, nw2, fin = (sh["npad"], sh["npc"], sh["nw"], sh["nw2"],
                               sh["fin"])
    lo_rows = sh["lo_rows"]
    caps = sh["caps"]
    seg_off_map = sh["seg_off_map"]
    groups_w = sh["groups_w"]
    n_nodes = sh["n_nodes"]
    Kd, Ks, Wd, Ws = sh["Kd"], sh["Ks"], sh["Wd"], sh["Ws"]
    nt = npad // 128          # node tiles
    kt = fin // 128           # contraction tiles for features @ W

    nc = bacc.Bacc("TRN2", target_bir_lowering=False, debug=False,
                   num_devices=CORES, num_swdge_queues=4)

    featT = nc.dram_tensor("featT", [fin, npad // 2], BF16,
                           kind="ExternalInput")
    w_mat = nc.dram_tensor("w_mat", [fin, OUT_FEAT], BF16, kind="ExternalInput")
    gam = nc.dram_tensor("gam", [1, OUT_FEAT], F32, kind="ExternalInput")
    bet = nc.dram_tensor("bet", [1, OUT_FEAT], F32, kind="ExternalInput")
    io8_d = nc.dram_tensor("io8", [128, OHB * 128], BF16, kind="ExternalInput")
    ident = nc.dram_tensor("ident", [128, 128], F32, kind="ExternalInput")
    ones_row = nc.dram_tensor("ones_row", [1, 128], F32, kind="ExternalInput")
    idx16 = nc.dram_tensor("idx16", [128, sh["totcap"] // 16], I16,
                           kind="ExternalInput")
    dstl = nc.dram_tensor("dstl", [128, sh["nch_d"]], F32, kind="ExternalInput")
    erd = nc.dram_tensor("erd", [128, sh["nch_d"]], F32, kind="ExternalInput")
    dtab = nc.dram_tensor("dtab", [nw2, Wd], F32, kind="ExternalInput")
    stab = nc.dram_tensor("stab", [nw2, Ws], F32, kind="ExternalInput")
    noder = nc.dram_tensor("noder", [npc, OUT_FEAT], F32, kind="ExternalInput")
    out = nc.dram_tensor("out", [npc, OUT_FEAT], F32, kind="ExternalOutput")

    hn = nc.dram_tensor("hn", [npad, OUT_FEAT], BF16, addr_space="Shared")
    barr_in = nc.dram_tensor("barr_in", [1, 128], F32)
    barr_out = nc.dram_tensor("barr_out", [1, 128], F32)
    degb_in = nc.dram_tensor("degb_in", [1, npc], F32)
    degb_out = nc.dram_tensor("degb_out", [nt, 128], F32)
    statb_in = nc.dram_tensor("statb_in", [1, 2 * OUT_FEAT], F32)
    statb_out = nc.dram_tensor("statb_out", [1, 2 * OUT_FEAT], F32)

    groups = [list(range(CORES))]
    pair_groups = [[2 * i, 2 * i + 1] for i in range(CORES // 2)]

    with tile.TileContext(nc) as tc:
        nc.gpsimd.load_library(library_config.mlp)
        with (
            tc.tile_pool(name="const", bufs=1) as cpool,
            tc.tile_pool(name="aux", bufs=1) as apool,
            tc.tile_pool(name="work", bufs=1) as wpool,
            tc.tile_pool(name="psum", bufs=1, space="PSUM") as pps,
        ):
            # ---------- constants ----------
            w_tiles = []
            for k in range(kt):
                wt = cpool.tile([128, OUT_FEAT], BF16, tag=f"wk{k}", name=f"wk{k}")
                nc.sync.dma_start(out=wt[:, :], in_=w_mat[k * 128:(k + 1) * 128, :])
                w_tiles.append(wt)
            io8_sb = cpool.tile([128, OHB * 128], BF16, tag="io8", name="io8")
            nc.sync.dma_start(out=io8_sb[:, :], in_=io8_d[:, :])
            idn = cpool.tile([128, 128], F32, tag="idn", name="idn")
            nc.sync.dma_start(out=idn[:, :], in_=ident[:, :])
            onesr = cpool.tile([1, 128], F32, tag="onesr", name="onesr")
            nc.sync.dma_start(out=onesr[:, :], in_=ones_row[:, :])
            ones16 = cpool.tile([128, 1], BF16, tag="ones16", name="ones16")
            nc.vector.memset(ones16[:, :], 1.0)
            gam_sb = cpool.tile([1, OUT_FEAT], F32, tag="gam_sb", name="gam_sb")
            nc.sync.dma_start(out=gam_sb[:, :], in_=gam[:, :])
            bet_sb = cpool.tile([1, OUT_FEAT], F32, tag="bet_sb", name="bet_sb")
            nc.sync.dma_start(out=bet_sb[:, :], in_=bet[:, :])

            # ---------- phase B: degrees via fixed-slot scans ----------
            with tc.tile_pool(name="scan", bufs=1) as spool, \
                    tc.tile_pool(name="scan_ps", bufs=1, space="PSUM") as sps:
                # deg_src -> degb_in -> AllGather
                st_sb = spool.tile([nw2, Ws], F32, tag="st_sb", name="st_sb")
                nc.sync.dma_start(out=st_sb[:, :], in_=stab[:, :])
                sk = spool.tile([nw2, Ws], F32, tag="sk", name="sk")
                nc.vector.tensor_scalar(sk[:, :], st_sb[:, :], P_EDGE, None,
                                        op0=OP.is_ge)
                ssc = spool.tile([nw2, Ws], F32, tag="ssc", name="ssc")
                nc.vector.tensor_tensor_scan(ssc[:, :], sk[:, :], sk[:, :],
                                             0.0, op0=OP.add, op1=OP.bypass)
                degs = spool.tile([nw2, 64], F32, tag="degs", name="degs")
                nc.vector.tensor_sub(degs[:, :], ssc[:, Ks::Ks],
                                     ssc[:, 0:64 * Ks:Ks])
                nc.sync.dma_start(
                    out=degb_in.ap().rearrange("o (p j) -> p (o j)", p=nw2),
                    in_=degs[:, :])
                if nocc:
                    nc.sync.dma_start(
                        out=degb_out[0:npc // 128, :],
                        in_=degb_in.ap().rearrange("o (r c) -> (o r) c", c=128))
                else:
                    nc.gpsimd.collective_compute(
                        "AllGather", OP.bypass, replica_groups=groups,
                        ins=[degb_in.ap().opt()], outs=[degb_out.ap().opt()])

                # deg_dst (local, no collective) -> d3T [128, nw]
                dt_sb = spool.tile([nw2, Wd], F32, tag="dt_sb", name="dt_sb")
                nc.sync.dma_start(out=dt_sb[:, :], in_=dtab[:, :])
                dk = spool.tile([nw2, Wd], F32, tag="dk", name="dk")
                nc.vector.tensor_scalar(dk[:, :], dt_sb[:, :], P_EDGE, None,
                                        op0=OP.is_ge)
                dsc = spool.tile([nw2, Wd], F32, tag="dsc", name="dsc")
                nc.vector.tensor_tensor_scan(dsc[:, :], dk[:, :], dk[:, :],
                                             0.0, op0=OP.add, op1=OP.bypass)
                degd = spool.tile([nw2, 64], F32, tag="degd", name="degd")
                nc.vector.tensor_sub(degd[:, :], dsc[:, Kd::Kd],
                                     dsc[:, 0:64 * Kd:Kd])
                tp64 = sps.tile([64, nw2], F32, tag="tp64", name="tp64")
                nc.tensor.transpose(tp64[:, 0:nw2], degd[0:nw2, 0:64],
                                    idn[0:nw2, 0:nw2])
                d1 = wpool.tile([128, nw], F32, tag="d1", name="d1")
                nc.vector.tensor_scalar_max(d1[0:64, :], tp64[:, 0::2], 1.0)
                nc.vector.tensor_scalar_max(d1[64:128, :], tp64[:, 1::2], 1.0)
                d2 = wpool.tile([128, nw], F32, tag="d2", name="d2")
                nc.scalar.sqrt(d2[:, :], d1[:, :])
                d3T = wpool.tile([128, nw], F32, tag="d3T", name="d3T")
                nc.vector.reciprocal(d3T[:, :], d2[:, :])

            # reload MY half's degrees -> [128, nt//2] rsqrt(max(deg,1))
            nt_half = nt // 2
            par = nc.sync.partition_id() % 2
            rbase = par * nt_half
            rdegs = wpool.tile([128, nt_half], F32, tag="rdegs", name="rdegs")
            with tc.tile_pool(name="degld", bufs=2) as dl_pool, \
                    tc.tile_pool(name="degt_ps", bufs=2, space="PSUM") as tps_pool:
                for blk in range((nt_half + 127) // 128):
                    r0 = blk * 128
                    r1 = min(r0 + 128, nt_half)
                    nrow = r1 - r0
                    dl = dl_pool.tile([128, 128], F32, tag="dl", name=f"dl{blk}")
                    nc.sync.dma_start(
                        out=dl[0:nrow, :],
                        in_=degb_out[bass.ds(rbase + r0, nrow), :])
                    tp = tps_pool.tile([128, 128], F32, tag="tp", name=f"tp{blk}")
                    nc.tensor.transpose(tp[:, 0:nrow], dl[0:nrow, :],
                                        idn[0:nrow, 0:nrow])
                    t1 = dl_pool.tile([128, 128], F32, tag="t1", name=f"t1{blk}")
                    nc.vector.tensor_scalar_max(t1[:, 0:nrow], tp[:, 0:nrow], 1.0)
                    t2 = dl_pool.tile([128, 128], F32, tag="t2", name=f"t2{blk}")
                    nc.scalar.sqrt(t2[:, 0:nrow], t1[:, 0:nrow])
                    nc.vector.reciprocal(rdegs[:, r0:r1], t2[:, 0:nrow])

            # ---------- phase C: hn table (my half, batched DMA) ----------
            NB = 7
            assert nt_half % NB == 0
            row_base = par * (nt_half * 128)
            hn_writes = []
            with tc.tile_pool(name="hload", bufs=4) as hl_pool, \
                    tc.tile_pool(name="hps", bufs=6, space="PSUM") as hps_pool, \
                    tc.tile_pool(name="hout", bufs=4) as ho_pool:
                for b in range(nt_half // NB):
                    n0 = b * NB * 128
                    ft = hl_pool.tile([128, kt * NB * 128], BF16, tag="ft",
                                      name=f"ft{b}")
                    nc.scalar.dma_start(
                        out=ft[:, :].rearrange("p (k n) -> p k n", k=kt),
                        in_=featT[:, n0:n0 + NB * 128].rearrange(
                            "(k p) n -> p k n", k=kt))
                    hnt = ho_pool.tile([128, NB * OUT_FEAT], BF16, tag="hnt",
                                       name=f"hnt{b}")
                    for j in range(NB):
                        hps = hps_pool.tile([128, OUT_FEAT], F32, tag="hps",
                                            name=f"hps{b}_{j}")
                        for k in range(kt):
                            nc.tensor.matmul(
                                hps[:, :],
                                lhsT=ft[:, (k * NB + j) * 128:
                                        (k * NB + j + 1) * 128],
                                rhs=w_tiles[k][:, :],
                                start=(k == 0), stop=(k == kt - 1))
                        t = b * NB + j
                        h_sl = hnt[:, j * OUT_FEAT:(j + 1) * OUT_FEAT]
                        if j % 2 == 0:
                            nc.scalar.activation(h_sl, hps[:, :], AF.Copy,
                                                 scale=rdegs[:, t:t + 1])
                        else:
                            nc.vector.tensor_scalar(h_sl, hps[:, :],
                                                    rdegs[:, t:t + 1], None,
                                                    op0=OP.mult)
                    wr = nc.sync.dma_start(
                        out=hn[bass.ds(row_base + n0, NB * 128), :].rearrange(
                            "(t p) f -> p t f", p=128),
                        in_=hnt[:, :].rearrange("p (t f) -> p t f", t=NB))
                    hn_writes.append(wr)
            # pair barrier: both halves of the shared table written
            bw = nc.sync.dma_start(out=barr_in[:, :], in_=onesr[:, :])
            for wr in hn_writes:
                _add_dep_helper(bw.ins, wr.ins, sync=True,
                                reason="hn writes before pair barrier")
            if nocc:
                barrier_cc = None
            else:
                barrier_cc = nc.gpsimd.collective_compute(
                    "AllReduce", OP.add, replica_groups=pair_groups,
                    ins=[barr_in.ap().opt()], outs=[barr_out.ap().opt()])

            # ---------- phase D: gather + segment-sum ----------
            idx_sb = apool.tile([128, sh["totcap"] // 16], I16, tag="idx_sb",
                                name="idx_sb")
            nc.sync.dma_start(out=idx_sb[:, :], in_=idx16[:, :])
            dstl_sb = apool.tile([128, sh["nch_d"]], F32, tag="dstl_sb",
                                 name="dstl_sb")
            nc.sync.dma_start(out=dstl_sb[:, :], in_=dstl[:, :])
            erd_sb = apool.tile([128, sh["nch_d"]], F32, tag="erd_sb",
                                name="erd_sb")
            nc.sync.dma_start(out=erd_sb[:, :], in_=erd[:, :])
            # dm = keep*(dstl+1)-1 (bf16, exact small ints; -1 for dropped/pad)
            keep_d = apool.tile([128, sh["nch_d"]], F32, tag="keep_d",
                                name="keep_d")
            nc.vector.tensor_scalar(keep_d[:, :], erd_sb[:, :], P_EDGE, None,
                                    op0=OP.is_ge)
            dmf = apool.tile([128, sh["nch_d"]], F32, tag="dmf", name="dmf")
            nc.vector.tensor_scalar(dmf[:, :], dstl_sb[:, :], 1.0, None,
                                    op0=OP.add)
            nc.vector.tensor_mul(dmf[:, :], dmf[:, :], keep_d[:, :])
            dm = apool.tile([128, sh["nch_d"]], BF16, tag="dm", name="dm")
            nc.vector.tensor_scalar(dm[:, :], dmf[:, :], 1.0, None,
                                    op0=OP.subtract)

            stat_ps = pps.tile([1, 2 * OUT_FEAT], F32, tag="stat_ps",
                               name="stat_ps")

            # node-dropout masks built here: phase D's DVE has slack
            inv_keep = 1.0 / (1.0 - P_NODE)
            msk_tiles = {}
            NBF = 7
            with tc.tile_pool(name="nrl", bufs=3) as nr_pool:
                for b in range((nw + NBF - 1) // NBF):
                    ws_m = list(range(b * NBF, min((b + 1) * NBF, nw)))
                    nb = len(ws_m)
                    n0 = ws_m[0] * 128
                    nrt = nr_pool.tile([128, NBF * OUT_FEAT], F32,
                                       tag="nrt", name=f"nrt{b}")
                    nc.sync.dma_start(
                        out=nrt[:, 0:nb * OUT_FEAT].rearrange(
                            "p (t f) -> p t f", t=nb),
                        in_=noder[n0:n0 + nb * 128, :].rearrange(
                            "(t p) f -> p t f", p=128))
                    for i, w in enumerate(ws_m):
                        msk = wpool.tile([128, OUT_FEAT], BF16,
                                         tag=f"msk{w}", name=f"msk{w}")
                        nc.vector.tensor_scalar(
                            msk[:, :],
                            nrt[:, i * OUT_FEAT:(i + 1) * OUT_FEAT],
                            P_NODE, inv_keep, op0=OP.is_ge, op1=OP.mult)
                        msk_tiles[w] = msk

            active = [w for w in range(nw) if caps[w, 0] + caps[w, 1] > 0]
            agg_tiles = {}
            max_gcap = max(
                (sum(int(caps[w, 0] + caps[w, 1]) for w in ws)
                 for ws in groups_w), default=128)
            hn_lo = hn[0:lo_rows, :]
            hn_hi = hn[lo_rows:npad, :]

            ngath = 0
            with tc.tile_pool(name="gath", bufs=4) as gpool, \
                    tc.tile_pool(name="mdst", bufs=8) as mpool, \
                    tc.tile_pool(name="aggps", bufs=4, space="PSUM") as aps_pool:
                for gidx, ws in enumerate(groups_w):
                    gcap = sum(int(caps[w, 0] + caps[w, 1]) for w in ws)
                    if gcap == 0:
                        continue
                    g0 = int(min(seg_off_map[w * 2 + r]
                                 for w in ws for r in range(2)
                                 if caps[w, r] > 0))
                    gt = gpool.tile([128, max_gcap], BF16, tag="gt",
                                    name=f"gt{gidx}")
                    gt3 = gt[:, 0:gcap].rearrange(
                        "p (c e) -> p c e", e=OUT_FEAT)
                    m8_of = {}
                    # one gather per (group, range), round-robin queues
                    for r, src_view in ((0, hn_lo), (1, hn_hi)):
                        rcap = sum(int(caps[w, r]) for w in ws)
                        if rcap == 0:
                            continue
                        roff = int(min(seg_off_map[w * 2 + r] for w in ws
                                       if caps[w, r] > 0)) - g0
                        gth = nc.gpsimd.dma_gather(
                            gt3[:, roff // 128:(roff + rcap) // 128, :],
                            src_view,
                            idx_sb[:, (g0 + roff) // 16:
                                   (g0 + roff + rcap) // 16],
                            rcap, rcap, OUT_FEAT,
                            single_packet=False,
                            queue_num=ngath % 4)
                        ngath += 1
                        if barrier_cc is not None:
                            _add_dep_helper(gth.ins, barrier_cc.ins,
                                            sync=True,
                                            reason="gather after barrier")
                        # batched one-hot builds for this (group, range):
                        # contiguous chunk cols [c0, c0+ncols)
                        c0 = (g0 + roff) // 128
                        ncols = rcap // 128
                        for qi in range((ncols + OHB - 1) // OHB):
                            wq = min(OHB, ncols - qi * OHB)
                            m8 = mpool.tile([128, OHB * 128], BF16, tag="m8",
                                            name=f"m8_{gidx}_{r}_{qi}")
                            cq = c0 + qi * OHB
                            nc.vector.tensor_tensor(
                                m8[:, 0:wq * 128].rearrange(
                                    "p (c f) -> p c f", c=wq),
                                io8_sb[:, 0:wq * 128].rearrange(
                                    "p (c f) -> p c f", c=wq),
                                dm[:, cq:cq + wq].broadcast_to((128, wq, 128)),
                                op=OP.is_equal)
                            for j in range(wq):
                                m8_of[cq + j] = (m8, j)
                    # consume chunks window-by-window
                    for w in ws:
                        nch_w = int(caps[w, 0] + caps[w, 1]) // 128
                        if nch_w == 0:
                            continue
                        chunk_cols = []
                        for r in range(2):
                            cap = int(caps[w, r])
                            for k in range(cap // 128):
                                chunk_cols.append(
                                    int(seg_off_map[w * 2 + r]) // 128 + k)
                        aps = aps_pool.tile([128, OUT_FEAT], F32,
                                            tag="aps", name=f"aps{w}")
                        for ki, col in enumerate(chunk_cols):
                            m8, j = m8_of[col]
                            lo_off = (col * 128 - g0)
                            nc.tensor.matmul(
                                aps[:, :],
                                lhsT=m8[:, j * 128:(j + 1) * 128],
                                rhs=gt[:, lo_off:lo_off + OUT_FEAT],
                                start=(ki == 0),
                                stop=(ki == len(chunk_cols) - 1))
                        # agg|sq packed in one tile for a single stats matmul
                        asq = wpool.tile([128, 2 * OUT_FEAT], BF16,
                                         tag=f"asq{w}", name=f"asq{w}")
                        nc.scalar.activation(
                            asq[:, 0:OUT_FEAT], aps[:, 0:OUT_FEAT],
                            AF.Copy, scale=d3T[:, w:w + 1])
                        nc.scalar.square(asq[:, OUT_FEAT:2 * OUT_FEAT],
                                         asq[:, 0:OUT_FEAT])
                        nc.tensor.matmul(stat_ps[0:1, :],
                                         lhsT=ones16[:, :], rhs=asq[:, :],
                                         start=(w == active[0]),
                                         stop=(w == active[-1]))
                        agg_tiles[w] = asq

            # ---------- phase E: BN stats + finale ----------
            stat_sb = wpool.tile([1, 2 * OUT_FEAT], F32, tag="stat_sb",
                                 name="stat_sb")
            nc.vector.tensor_copy(stat_sb[:, :], stat_ps[:, :])
            nc.sync.dma_start(out=statb_in[:, :], in_=stat_sb[:, :])
            if nocc:
                nc.sync.dma_start(out=statb_out[:, :], in_=statb_in[:, :])
            else:
                nc.gpsimd.collective_compute(
                    "AllReduce", OP.add, replica_groups=groups,
                    ins=[statb_in.ap().opt()], outs=[statb_out.ap().opt()])
            stat2 = wpool.tile([1, 2 * OUT_FEAT], F32, tag="stat2",
                               name="stat2")
            nc.sync.dma_start(out=stat2[:, :], in_=statb_out[:, :])

            inv_n = 1.0 / float(n_nodes)
            mn = wpool.tile([1, OUT_FEAT], F32, tag="mn", name="mn")
            nc.vector.tensor_scalar(mn[:, :], stat2[0:1, 0:OUT_FEAT], inv_n,
                                    None, op0=OP.mult)
            ex2 = wpool.tile([1, OUT_FEAT], F32, tag="ex2", name="ex2")
            nc.vector.tensor_scalar(ex2[:, :], stat2[0:1, OUT_FEAT:2 * OUT_FEAT],
                                    inv_n, None, op0=OP.mult)
            var = wpool.tile([1, OUT_FEAT], F32, tag="var", name="var")
            nc.vector.tensor_mul(var[:, :], mn[:, :], mn[:, :])
            nc.vector.tensor_sub(var[:, :], ex2[:, :], var[:, :])
            nc.vector.tensor_scalar_add(var[:, :], var[:, :], BN_EPS)
            sd = wpool.tile([1, OUT_FEAT], F32, tag="sd", name="sd")
            nc.scalar.sqrt(sd[:, :], var[:, :])
            istd = wpool.tile([1, OUT_FEAT], F32, tag="istd", name="istd")
            nc.vector.reciprocal(istd[:, :], sd[:, :])
            st_row = wpool.tile([1, 2 * OUT_FEAT], F32, tag="st_row",
                                name="st_row")
            # s = gamma * istd ; t = beta - mean * s
            nc.vector.tensor_mul(st_row[0:1, 0:OUT_FEAT], gam_sb[:, :],
                                 istd[:, :])
            tmp_t = wpool.tile([1, OUT_FEAT], F32, tag="tmp_t", name="tmp_t")
            nc.vector.tensor_mul(tmp_t[:, :], mn[:, :],
                                 st_row[0:1, 0:OUT_FEAT])
            nc.vector.tensor_sub(st_row[0:1, OUT_FEAT:2 * OUT_FEAT],
                                 bet_sb[:, :], tmp_t[:, :])
            with tc.tile_pool(name="bps", bufs=1, space="PSUM") as bps_pool:
                bps = bps_pool.tile([128, 2 * OUT_FEAT], F32, tag="bps",
                                    name="bps")
                nc.tensor.matmul(bps[:, :], lhsT=onesr[:, :], rhs=st_row[:, :],
                                 start=True, stop=True)
                st_bc = wpool.tile([128, 2 * OUT_FEAT], F32, tag="st_bc",
                                   name="st_bc")
                nc.vector.tensor_copy(st_bc[:, :], bps[:, :])

            with tc.tile_pool(name="fin", bufs=3) as fpool:
                for b in range((nw + NBF - 1) // NBF):
                    ws = list(range(b * NBF, min((b + 1) * NBF, nw)))
                    nb = len(ws)
                    n0 = ws[0] * 128
                    ot = fpool.tile([128, NBF * OUT_FEAT], F32, tag="ot",
                                    name=f"ot{b}")
                    for i, w in enumerate(ws):
                        agg = agg_tiles.get(w)
                        o_sl = ot[:, i * OUT_FEAT:(i + 1) * OUT_FEAT]
                        y = fpool.tile([128, OUT_FEAT], F32, tag="y",
                                       name=f"y{w}")
                        if agg is None:
                            nc.vector.tensor_copy(
                                y[:, :], st_bc[:, OUT_FEAT:2 * OUT_FEAT])
                        else:
                            nc.vector.tensor_mul(y[:, :], agg[:, 0:OUT_FEAT],
                                                 st_bc[:, 0:OUT_FEAT])
                            nc.vector.tensor_add(
                                y[:, :], y[:, :],
                                st_bc[:, OUT_FEAT:2 * OUT_FEAT])
                        nc.gpsimd.tensor_mul(o_sl, y[:, :], msk_tiles[w][:, :])
                    nc.scalar.dma_start(
                        out=out[n0:n0 + nb * 128, :].rearrange(
                            "(t p) f -> p t f", p=128),
                        in_=ot[:, 0:nb * OUT_FEAT].rearrange(
                            "p (t f) -> p t f", t=nb))

    nc.compile()
    return nc


_CACHE = {}


def _get_program(inputs):
    key = tuple(np.asarray(inputs["src"])[:8].tolist()) + (
        len(np.asarray(inputs["src"])),)
    if key not in _CACHE:
        sh, in_maps = prep_inputs(
            inputs["features"], inputs["W"], inputs["gamma"], inputs["beta"],
            inputs["src"], inputs["dst"], inputs["edge_rand"],
            inputs["node_rand"])
        nc = build_program(sh)
        _CACHE[key] = (sh, nc)
    else:
        sh, nc = _CACHE[key]
        _, in_maps = prep_inputs(
            inputs["features"], inputs["W"], inputs["gamma"], inputs["beta"],
            inputs["src"], inputs["dst"], inputs["edge_rand"],
            inputs["node_rand"])
    return sh, _CACHE[key][1], in_maps


def kernel(**inputs):
    sh, nc, in_maps = _get_program(inputs)
    res = run_bass_kernel_spmd(nc, in_maps, core_ids=list(range(CORES)))
    npc = sh["npc"]
    full = np.concatenate([res.results[c]["out"] for c in range(CORES)],
                          axis=0)
    return np.ascontiguousarray(full[:sh["n_nodes"]]).astype(np.float32)
